# revision 9
# baseline (speedup 1.0000x reference)
"""Trainium2 Bass kernel for nn_Attention_32839319945876 (sparse_attention).

Head-parallel: 48 heads -> 6 per core on 8 NeuronCores, pair-packed so the
two branch-siblings of each base head share pre-wedge keys. All per-core
inputs ship as ONE packed blob (per-arg dispatch overhead dominates call
cost here): f32 regions (bitcast-packed) for the selection-critical
pre-score path, fp16 for post-softmax weights. A/X ship T-sharded and are
AllGathered on device; K wedge is applied on device. Device does
projections (f32r), wedge+rope (f32), exp-softmax stats, top-12 threshold
via chunked max8, masked-exp weight matrix (bf16), marker matmul, MLP, and
per-core partial output projection combined by ReduceScatter.
"""

import math
import sys
import types

import numpy as np

try:
    import antenv.axon_hooks  # noqa: F401
except Exception:
    _m = types.ModuleType("antenv.axon_hooks")
    _m.get_axon_ntff_profile_hook = lambda: None
    sys.modules["antenv.axon_hooks"] = _m

import contextlib

import concourse.bass as bass  # noqa: F401
import concourse.bacc as bacc
import concourse.tile as tile
from concourse import mybir
from concourse.bass_utils import run_bass_kernel_spmd

B, T, C = 1, 1024, 768
N_HEAD = 12
N_BR = 4
DH = C // N_HEAD          # 64
H_TOT = N_BR * N_HEAD     # 48
K_RET = 12
EPS = 1.1920929e-07
N_CORES = 8
HPC = H_TOT // N_CORES    # 6
NPAIR = HPC // 2          # 3
NTB = T // 128            # 8
NCH = C // 128            # 6
SCALE = DH ** -0.5
SIG_C = math.pi / math.sqrt(3.0)
NEG = -40.0

F32 = mybir.dt.float32
F32R = mybir.dt.float32r
F16 = mybir.dt.float16
BF16 = mybir.dt.bfloat16
ACTF = mybir.ActivationFunctionType
ALU = mybir.AluOpType

_DUPCOL = np.concatenate([np.arange(0, DH, 2), np.arange(1, DH, 2),
                          np.arange(0, DH, 2), np.arange(1, DH, 2)])

# ---- packed blob [772, 3072] fp16; f32 blocks stored as fp16 bit-pairs.
# name -> (row, fp16_col, nrows, n_elems, dtype)
BLOB_R, BLOB_C = 772, 3072
AX_ROWS = 128
BLOCKS = {
    "AX":    (0,    0,    128, 1536, F32),  # 6 A chunks then 6 X chunks
    "WQ03":  (128,  0,    128, 1536, F32),  # WQ chunks 0..3 (4x384)
    "WQ45":  (256,  0,    128, 768,  F32),  # WQ chunks 4,5
    "WK03":  (256,  1536, 128, 768,  F32),  # WK chunks 0..3 (4x192)
    "WK45":  (384,  0,    128, 384,  F32),  # WK chunks 4,5
    "Mq":    (384,  768,  128, 384,  F32),
    "Mk":    (384,  1536, 128, 384,  F32),
    "Padd":  (384,  2304, 128, 64,   F16),
    "tri":   (384,  2432, 128, 128,  F16),
    "ident": (384,  2560, 128, 128,  F16),
    "esink": (384,  2688, 128, 8,    F32),
    "Frope2": (512, 0,    64,  1024, F32),  # [cos.T; sin.T]
    "fcw":   (512,  2048, 64,  256,  F16),
    "pjw0":  (512,  2304, 128, 64,   F16),
    "pjw1":  (512,  2368, 128, 64,   F16),
    "WOp0":  (640,  0,    128, 768,  F16),
    "WOp1":  (640,  768,  128, 768,  F16),
    "WOp2":  (640,  1536, 128, 768,  F16),
    "WQb":   (768,  0,    1,   384,  F16),
    "WKb":   (768,  384,  1,   192,  F16),
    "pjb":   (768,  576,  1,   64,   F16),
    "vsink": (768,  640,  1,   384,  F16),
    "fcb":   (768,  1024, 1,   256,  F16),
    "yb":    (768,  1280, 1,   768,  F16),
}


def core_heads(core):
    """6 heads per core; heads (2j, 2j+1) share base bases[j]."""
    p = core // 2
    bases = [3 * p, 3 * p + 1, 3 * p + 2]
    brs = (0, 1) if core % 2 == 0 else (2, 3)
    heads = []
    for b in bases:
        for br in brs:
            heads.append(br * N_HEAD + b)
    return heads, bases


def _rope_half():
    inv = 1.0 / (10000.0 ** (np.arange(0, DH, 2, dtype=np.float64) / DH))
    ang = np.arange(T, dtype=np.float64)[:, None] * inv[None, :]
    return np.concatenate([np.cos(ang).T, np.sin(ang).T], axis=0)  # [64, T]


def _pair_add():
    P = np.zeros((128, DH), np.float64)
    for m in range(32):
        P[m, m] = 1.0
        P[m + 32, m] = 1.0
    for m in range(32, 64):
        P[m + 32, m] = 1.0
        P[m + 64, m] = 1.0
    return P


def _host_prep(A, X, WK_w, WK_b, WQ_w, WQ_b, wedge_A, wedge_bias, sink,
               v_nulls, fc_w, fc_b, proj_w, proj_b, WO, WO_b):
    A = np.asarray(A, np.float32)
    X = np.asarray(X, np.float32)
    WK_w = np.asarray(WK_w); WK_b = np.asarray(WK_b)
    WQ_w = np.asarray(WQ_w); WQ_b = np.asarray(WQ_b)
    wedge_A = np.asarray(wedge_A); wedge_bias = np.asarray(wedge_bias)
    sink = np.asarray(sink); v_nulls = np.asarray(v_nulls)
    fc_w = np.asarray(fc_w); fc_b = np.asarray(fc_b)
    proj_w = np.asarray(proj_w); proj_b = np.asarray(proj_b)
    WO = np.asarray(WO); WO_b = np.asarray(WO_b)

    Askew = (wedge_A - wedge_A.T).astype(np.float64)
    F2 = _rope_half()
    Padd = _pair_add()
    tri = np.where(np.tril(np.ones((128, 128), bool)), 0.0, NEG)
    ident = np.eye(128)
    vre = v_nulls.reshape(H_TOT, DH)
    esink_all = np.exp(sink.astype(np.float64))
    yb_mean = WO_b.mean(0)

    in_maps = []
    for core in range(N_CORES):
        heads, bases = core_heads(core)
        blob = np.zeros((BLOB_R, BLOB_C), np.float16)

        def put(name, arr):
            r, c, h, w, dt_ = BLOCKS[name]
            if dt_ == F32:
                a = np.ascontiguousarray(np.asarray(arr, np.float32))
                assert a.shape == (h, w), (name, a.shape, (h, w))
                blob[r:r + h, c:c + 2 * w] = a.view(np.float16)
            else:
                a = np.asarray(arr, np.float16)
                assert a.shape == (h, w), (name, a.shape, (h, w))
                blob[r:r + h, c:c + w] = a

        t0 = core * 128
        ax = np.zeros((128, 1536), np.float32)
        for c in range(NCH):
            ax[:, c * 128:(c + 1) * 128] = A[0][t0:t0 + 128,
                                                c * 128:(c + 1) * 128].T
            ax[:, 768 + c * 128:768 + (c + 1) * 128] = \
                X[0][t0:t0 + 128, c * 128:(c + 1) * 128].T
        put("AX", ax)

        WQ6 = np.concatenate([WQ_w[:, h * DH:(h + 1) * DH] for h in heads], 1)
        wq03 = np.zeros((128, 1536), np.float32)
        for c in range(4):
            wq03[:, c * 384:(c + 1) * 384] = WQ6[c * 128:(c + 1) * 128]
        put("WQ03", wq03)
        put("WQ45", np.concatenate([WQ6[4 * 128:5 * 128],
                                    WQ6[5 * 128:6 * 128]], axis=1))
        WK3 = np.concatenate([WK_w[:, b * DH:(b + 1) * DH] for b in bases], 1)
        put("WK03", np.concatenate([WK3[c * 128:(c + 1) * 128]
                                    for c in range(4)], axis=1))
        put("WK45", np.concatenate([WK3[4 * 128:5 * 128],
                                    WK3[5 * 128:6 * 128]], axis=1))

        Ms = {h: np.eye(DH) + Askew + np.diag(wedge_bias[h].astype(np.float64))
              for h in heads}
        mq = np.zeros((128, 384), np.float64)
        mk = np.zeros((128, 384), np.float64)
        for j in range(NPAIR):
            h0, h1 = heads[2 * j], heads[2 * j + 1]
            mq[0:64, j * 128:(j + 1) * 128] = (Ms[h0] * SCALE)[:, _DUPCOL]
            mq[64:128, j * 128:(j + 1) * 128] = (Ms[h1] * SCALE)[:, _DUPCOL]
            mk[0:64, j * 128:(j + 1) * 128] = Ms[h0][:, _DUPCOL]
            mk[64:128, j * 128:(j + 1) * 128] = Ms[h1][:, _DUPCOL]
        put("Mq", mq)
        put("Mk", mk)
        put("Padd", Padd)
        put("tri", tri)
        put("ident", ident)
        es = np.zeros((128, 8), np.float32)
        es[:, 0:HPC] = np.tile(
            np.array([esink_all[h] for h in heads], np.float32)[None, :],
            (128, 1))
        put("esink", es)
        put("Frope2", F2)
        put("fcw", fc_w)
        put("pjw0", proj_w[0:128])
        put("pjw1", proj_w[128:256])
        for j in range(NPAIR):
            h0, h1 = heads[2 * j], heads[2 * j + 1]
            s0 = WO[h0 // N_HEAD][(h0 % N_HEAD) * DH:(h0 % N_HEAD + 1) * DH]
            s1 = WO[h1 // N_HEAD][(h1 % N_HEAD) * DH:(h1 % N_HEAD + 1) * DH]
            put(f"WOp{j}", np.concatenate([s0, s1], 0) * 0.25)
        put("WQb", np.concatenate(
            [WQ_b[h * DH:(h + 1) * DH] for h in heads]).reshape(1, -1))
        put("WKb", np.concatenate(
            [WK_b[b * DH:(b + 1) * DH] for b in bases]).reshape(1, -1))
        put("pjb", proj_b.reshape(1, -1))
        put("vsink", np.concatenate(
            [vre[h] * esink_all[h] for h in heads]).reshape(1, -1))
        put("fcb", fc_b.reshape(1, -1))
        put("yb", (yb_mean if core == 0 else np.zeros(C)).reshape(1, -1))

        in_maps.append({"blob": blob})
    return in_maps


def build_kernel():
    nc = bacc.Bacc(target_bir_lowering=False, debug=False)
    blob = nc.declare_dram_parameter("blob", [BLOB_R, BLOB_C], F16,
                                     isOutput=False)
    out = nc.declare_dram_parameter("out", [C // N_CORES, T], F16,
                                    isOutput=True)
    ax_in = nc.dram_tensor("ax_in", [AX_ROWS, BLOB_C], F16)
    axg = nc.dram_tensor("axg", [N_CORES, AX_ROWS, BLOB_C], F16)
    y_bounce = nc.dram_tensor("y_bounce", [C, T], F32)
    y_rs = nc.dram_tensor("y_rs", [C // N_CORES, T], F32)

    def bap(name, dtype=None):
        r, c, h, w, dt_ = BLOCKS[name]
        if dt_ == F32:
            ap = blob[r:r + h, c:c + 2 * w]
            return ap.bitcast(dtype or F32)
        return blob[r:r + h, c:c + w]

    with tile.TileContext(nc) as tc:
        ctx = contextlib.ExitStack()
        with ctx:
            cpool = ctx.enter_context(tc.tile_pool(name="consts", bufs=1))
            wpool = ctx.enter_context(tc.tile_pool(name="weights", bufs=1))
            persist = ctx.enter_context(tc.tile_pool(name="persist", bufs=1))
            work = ctx.enter_context(tc.tile_pool(name="work", bufs=2))
            ework = ctx.enter_context(tc.tile_pool(name="ework", bufs=2))
            tiny = ctx.enter_context(tc.tile_pool(name="tiny", bufs=4))

            # ---------------- AllGather A/X shards ---------------------
            # collectives cannot read IO tensors: bounce via SBUF first
            axb = cpool.tile([AX_ROWS, BLOB_C], F16, name="axb")
            nc.sync.dma_start(axb[:], blob[0:AX_ROWS, 0:BLOB_C])
            nc.sync.dma_start(ax_in.ap(), axb[:])
            nc.gpsimd.collective_compute(
                "AllGather", ALU.bypass,
                ins=[ax_in.ap().opt()],
                outs=[axg.ap().opt()],
                replica_groups=[list(range(N_CORES))],
            )

            def load(pool, name, dtype=None, tag=None):
                r, c, h, w, dt_ = BLOCKS[name]
                dtype = dtype or dt_
                kw = {"tag": tag} if tag else {}
                t = pool.tile([h, w], dtype, name=name, **kw)
                nc.sync.dma_start(t[:], bap(name, dtype))
                return t

            ident16 = load(cpool, "ident")
            ident_b = cpool.tile([128, 128], BF16)
            nc.scalar.copy(ident_b[:], ident16[:])
            ident_f = cpool.tile([128, 128], F32)
            nc.scalar.copy(ident_f[:], ident16[:])
            tri_sb = load(cpool, "tri")
            esink_sb = load(cpool, "esink")
            # rope table [cos; -sin; sin; cos] built from the [cos; sin] half
            frope_sb = cpool.tile([128, T], F32, name="frope")
            r, c, _, _, _ = BLOCKS["Frope2"]
            f2 = blob[r:r + 64, c:c + 2048].bitcast(F32)
            nc.sync.dma_start(frope_sb[0:64, :], f2)
            nc.sync.dma_start(frope_sb[64:96, :],
                              blob[r + 32:r + 64, c:c + 2048].bitcast(F32))
            nc.sync.dma_start(frope_sb[96:128, :],
                              blob[r:r + 32, c:c + 2048].bitcast(F32))
            nc.vector.tensor_scalar(frope_sb[32:64, :], frope_sb[32:64, :],
                                    -1.0, None, ALU.mult)
            mq_sb = load(wpool, "Mq")
            mk_sb = load(wpool, "Mk")
            padd16 = load(wpool, "Padd")
            padd_sb = wpool.tile([128, DH], F32, name="padd_f")
            nc.scalar.copy(padd_sb[:], padd16[:])
            wqb_b = load(wpool, "WQb")
            wkb_b = load(wpool, "WKb")
            vsink_b = load(wpool, "vsink")
            fcw_b = load(wpool, "fcw")
            fcb_b = load(wpool, "fcb")
            pjw_b = [load(wpool, f"pjw{u}") for u in range(2)]
            pjb_b = load(wpool, "pjb")
            wop_b = [load(wpool, f"WOp{p}") for p in range(NPAIR)]
            yb_b = load(wpool, "yb")
            ones16 = cpool.tile([1, T], F16)
            nc.vector.memset(ones16[:], 1.0)

            # ---------------- stage B: projections + transposes --------
            # kp_slab packs [k | 1] per base: 3 blocks of 65 cols
            kp_slab = [persist.tile([128, NPAIR * (DH + 1)], BF16,
                                    name=f"kp{tb}") for tb in range(NTB)]
            qkpool_cm = tc.tile_pool(name="qkpool", bufs=1)
            qkpool = qkpool_cm.__enter__()
            actpool_cm = tc.tile_pool(name="actpool", bufs=1)
            actpool = actpool_cm.__enter__()
            qT = [qkpool.tile([128, T], F32, name=f"qTs{p}")
                  for p in range(NPAIR)]
            kT = [qkpool.tile([128, T], F32, name=f"kTs{p}")
                  for p in range(NPAIR)]

            def load_wq(cc):
                t = qkpool.tile([128, HPC * DH], F32R, name=f"wq{cc}",
                                tag=f"wx{cc}")
                if cc < 4:
                    r, c, _, _, _ = BLOCKS["WQ03"]
                    src = blob[r:r + 128,
                               c + cc * 768:c + (cc + 1) * 768].bitcast(F32R)
                else:
                    r, c, _, _, _ = BLOCKS["WQ45"]
                    u = cc - 4
                    src = blob[r:r + 128,
                               c + u * 768:c + (u + 1) * 768].bitcast(F32R)
                nc.sync.dma_start(t[:], src)
                return t

            def load_wk(cc):
                t = qkpool.tile([128, NPAIR * DH], F32R, name=f"wk{cc}",
                                tag=f"wx{cc}")
                if cc < 4:
                    r, c, _, _, _ = BLOCKS["WK03"]
                    src = blob[r:r + 128,
                               c + cc * 384:c + (cc + 1) * 384].bitcast(F32R)
                else:
                    r, c, _, _, _ = BLOCKS["WK45"]
                    u = cc - 4
                    src = blob[r:r + 128,
                               c + u * 384:c + (u + 1) * 384].bitcast(F32R)
                nc.sync.dma_start(t[:], src)
                return t

            def load_act(cc, xoff, nm):
                t = actpool.tile([128, T], F32R, name=nm, tag=f"act{cc}")
                for tb in range(NTB):
                    nc.sync.dma_start(
                        t[:, tb * 128:(tb + 1) * 128],
                        axg[tb, 0:128,
                            xoff + cc * 256:xoff + (cc + 1) * 256
                            ].bitcast(F32R))
                return t

            with tc.tile_pool(name="ps_b", bufs=2, space="PSUM") as ps_b:
                # ---- q pass ----
                wq_sb = [load_wq(c) for c in range(NCH)]
                a_sb = [load_act(c, 0, f"at{c}") for c in range(NCH)]
                for tb in range(NTB):
                    ts_ = slice(tb * 128, (tb + 1) * 128)
                    q_ps = ps_b.tile([128, HPC * DH], F32, tag="proj",
                                     name="q_ps")
                    for c in range(NCH):
                        nc.tensor.matmul(q_ps[:], a_sb[c][:, ts_],
                                         wq_sb[c][:],
                                         start=(c == 0), stop=False)
                    nc.tensor.matmul(q_ps[:], ones16[:, 0:128], wqb_b[:],
                                     start=False, stop=True)
                    q2 = work.tile([128, HPC * DH], F32, tag="q2", name="q2")
                    nc.scalar.activation(q2[:], q_ps[:], ACTF.Square)
                    ssq = tiny.tile([128, HPC], F32, tag="ssq", name="ssq")
                    nc.vector.reduce_sum(
                        ssq[:], q2[:].rearrange("p (h d) -> p h d", d=DH),
                        axis=mybir.AxisListType.X)
                    nc.vector.tensor_scalar(ssq[:], ssq[:], 1.0 / DH, EPS,
                                            ALU.mult, ALU.add)
                    nc.scalar.activation(ssq[:], ssq[:], ACTF.Sqrt)
                    rin = tiny.tile([128, HPC], F32, tag="rin", name="rin")
                    nc.vector.reciprocal(rin[:], ssq[:])
                    qs = work.tile([128, HPC * DH], F32, tag="qs", name="qs")
                    for h in range(HPC):
                        hsl = slice(h * DH, (h + 1) * DH)
                        nc.vector.tensor_scalar(qs[:, hsl], q_ps[:, hsl],
                                                rin[:, h:h + 1], None,
                                                ALU.mult)
                    for hh in range(HPC):
                        cs = slice(hh * DH, (hh + 1) * DH)
                        rs_ = slice((hh % 2) * DH, (hh % 2) * DH + DH)
                        tp = ps_b.tile([DH, 128], F32, tag="tp", name="tp")
                        nc.tensor.transpose(tp[:], qs[:, cs], ident_f[:])
                        nc.vector.tensor_copy(qT[hh // 2][rs_, ts_], tp[:])
                # ---- k pass (pre-wedge only; wedge applied in stage D) ----
                x_sb = [load_act(c, 1536, f"xt{c}") for c in range(NCH)]
                wk_sb = [load_wk(c) for c in range(NCH)]
                for tb in range(NTB):
                    ts_ = slice(tb * 128, (tb + 1) * 128)
                    kp_ps = ps_b.tile([128, NPAIR * DH], F32, tag="projk",
                                      name="kp_ps")
                    for c in range(NCH):
                        nc.tensor.matmul(kp_ps[:], x_sb[c][:, ts_],
                                         wk_sb[c][:],
                                         start=(c == 0), stop=False)
                    nc.tensor.matmul(kp_ps[:], ones16[:, 0:128], wkb_b[:],
                                     start=False, stop=True)
                    kpf = work.tile([128, NPAIR * DH], F32, tag="kpf",
                                    name="kpf")
                    nc.scalar.copy(kpf[:], kp_ps[:])
                    nc.vector.memset(kp_slab[tb][:], 1.0)
                    for b in range(NPAIR):
                        nc.scalar.copy(
                            kp_slab[tb][:, b * (DH + 1):b * (DH + 1) + DH],
                            kp_ps[:, b * DH:(b + 1) * DH])
                    for b in range(NPAIR):
                        tpk = ps_b.tile([DH, 128], F32, tag="tp", name="tpk")
                        nc.tensor.transpose(
                            tpk[:], kpf[:, b * DH:(b + 1) * DH], ident_f[:])
                        nc.vector.tensor_copy(kT[b][0:DH, ts_], tpk[:])
                        nc.vector.tensor_copy(kT[b][DH:128, ts_], tpk[:])
            actpool_cm.__exit__(None, None, None)

            # ---------------- stage D: wedge + rope --------------------
            qTr = [persist.tile([128, T], F32R, name=f"qTr{p}")
                   for p in range(NPAIR)]
            kTr = [persist.tile([128, T], F32R, name=f"kTr{p}")
                   for p in range(NPAIR)]
            with tc.tile_pool(name="ps_d", bufs=2, space="PSUM") as ps_d:
                for h in range(HPC):
                    pair, half = h // 2, h % 2
                    rs_ = slice(half * DH, half * DH + DH)
                    msl = slice(pair * 128, (pair + 1) * 128)
                    for (src, lhs, dst) in ((qT, mq_sb, qTr),
                                            (kT, mk_sb, kTr)):
                        xd = ps_d.tile([128, T], F32, tag="xd", name="xd")
                        for nh in range(2):
                            ns = slice(nh * 512, (nh + 1) * 512)
                            nc.tensor.matmul(xd[:, ns], lhs[rs_, msl],
                                             src[pair][rs_, ns],
                                             start=True, stop=True)
                        xr = work.tile([128, T], F32, tag="xrope", name="xr")
                        nc.vector.tensor_tensor(xr[:], xd[:], frope_sb[:],
                                                ALU.mult)
                        rr = ps_d.tile([DH, T], F32, tag="rr", bufs=1,
                                       name="rr")
                        for nh in range(2):
                            ns = slice(nh * 512, (nh + 1) * 512)
                            nc.tensor.matmul(rr[:, ns], padd_sb[:],
                                             xr[:, ns],
                                             start=True, stop=True)
                        nc.scalar.copy(dst[pair][rs_, :], rr[:])
            qkpool_cm.__exit__(None, None, None)

            # ---------------- stage E: per-head attention --------------
            ctx_slab = [persist.tile([128, T], F16, name=f"ctx{p}")
                        for p in range(NPAIR)]
            with (
                tc.tile_pool(name="ps_e1", bufs=1, space="PSUM") as ps_e1,
                tc.tile_pool(name="ps_e2", bufs=2, space="PSUM") as ps_e2,
                tc.tile_pool(name="ps_e3", bufs=1, space="PSUM") as ps_e3,
            ):
                for h in range(HPC):
                    pair, half = h // 2, h % 2
                    rs_ = slice(half * DH, half * DH + DH)
                    hsl = slice(h * DH, (h + 1) * DH)
                    for i in range(NTB):
                        L = (i + 1) * 128
                        ts_ = slice(i * 128, (i + 1) * 128)
                        s_ps = ps_e1.tile([128, 1024], F32, tag="s_ps",
                                          name="s_ps")
                        for n0 in range(0, L, 512):
                            n1 = min(n0 + 512, L)
                            nc.tensor.matmul(s_ps[:, n0:n1],
                                             qTr[pair][rs_, ts_],
                                             kTr[pair][rs_, n0:n1],
                                             start=True, stop=True)
                        nc.vector.tensor_tensor(s_ps[:, ts_], s_ps[:, ts_],
                                                tri_sb[:], ALU.add)
                        e_sb = ework.tile([128, 1024], F32, tag="e_sb",
                                          name="e_sb")
                        zrow = tiny.tile([128, 1], F32, tag="zrow",
                                         name="zrow")
                        nc.scalar.activation(e_sb[:, 0:L], s_ps[:, 0:L],
                                             ACTF.Exp, accum_out=zrow[:])
                        m8a = tiny.tile([128, 8], F32, tag="m8a", name="m8a")
                        m8b = tiny.tile([128, 8], F32, tag="m8b", name="m8b")
                        nc.vector.max(m8a[:], e_sb[:, 0:L])
                        r1f = ework.tile([128, 1024], F32, tag="r1f",
                                         name="r1f")
                        nc.vector.match_replace(r1f[:, 0:L], m8a[:],
                                                e_sb[:, 0:L], 0.0)
                        nc.vector.max(m8b[:], r1f[:, 0:L])
                        th_f = m8b[:, 3:4]
                        w_sb = ework.tile([128, 1024], BF16, tag="w_sb",
                                          name="w_sb")
                        msk = ework.tile([128, 1024], BF16, tag="msk",
                                         name="msk")
                        nc.gpsimd.tensor_scalar(msk[:, 0:L], e_sb[:, 0:L],
                                                th_f, None, ALU.is_ge)
                        nc.gpsimd.tensor_tensor(w_sb[:, 0:L], e_sb[:, 0:L],
                                                msk[:, 0:L], ALU.mult)
                        mk_ps = ps_e3.tile([128, DH + 1], F32, tag="mk_ps",
                                           name="mk_ps")
                        for j in range(i + 1):
                            js = slice(j * 128, (j + 1) * 128)
                            wt_ps = ps_e2.tile([128, 128], BF16, tag="sm",
                                               name="wt_ps")
                            nc.tensor.transpose(wt_ps[:], w_sb[:, js],
                                                ident_b[:])
                            wt_sb = ework.tile([128, 128], BF16, tag="wt_sb",
                                               name="wt_sb")
                            nc.scalar.copy(wt_sb[:], wt_ps[:])
                            nc.tensor.matmul(
                                mk_ps[:], wt_sb[:],
                                kp_slab[j][:, pair * (DH + 1):
                                           (pair + 1) * (DH + 1)],
                                start=(j == 0), stop=(j == i))
                        zf = tiny.tile([128, 1], F32, tag="zf", name="zf")
                        nc.vector.tensor_scalar(zf[:], zrow[:],
                                                esink_sb[:, h:h + 1],
                                                None, ALU.add)
                        den = tiny.tile([128, 1], F32, tag="den", name="den")
                        nc.vector.scalar_tensor_tensor(
                            den[:], zf[:], 1e-9, mk_ps[:, DH:DH + 1],
                            ALU.mult, ALU.add)
                        nu = tiny.tile([128, 1], F32, tag="nu", name="nu")
                        nc.vector.reciprocal(nu[:], den[:])
                        rz = tiny.tile([128, 1], F32, tag="rz", name="rz")
                        nc.vector.reciprocal(rz[:], zf[:])
                        mkn = tiny.tile([128, DH], F16, tag="mkn",
                                        name="mkn")
                        nc.vector.tensor_scalar(mkn[:], mk_ps[:, 0:DH],
                                                nu[:], None, ALU.mult)
                        mt_ps = ps_e2.tile([DH, 128], F16, tag="sm",
                                           name="mt_ps")
                        nc.tensor.transpose(mt_ps[:], mkn[:], ident16[:])
                        mknT = tiny.tile([DH, 128], F16, tag="mknT",
                                         name="mknT")
                        nc.scalar.copy(mknT[:], mt_ps[:])
                        h_ps = ps_e3.tile([128, 4 * DH], F32, tag="h_ps",
                                          name="h_ps")
                        nc.tensor.matmul(h_ps[:], mknT[:], fcw_b[:],
                                         start=True, stop=False)
                        nc.tensor.matmul(h_ps[:], ones16[:, 0:128], fcb_b[:],
                                         start=False, stop=True)
                        t1 = work.tile([128, 4 * DH], F16, tag="t1",
                                       name="t1")
                        nc.vector.tensor_scalar(t1[:], h_ps[:], 0.75, 1.0,
                                                ALU.mult, ALU.add)
                        hsq = work.tile([128, 4 * DH], F16, tag="hsq",
                                        name="hsq")
                        nc.scalar.activation(hsq[:], h_ps[:], ACTF.Square)
                        g = work.tile([128, 4 * DH], F16, tag="g", name="g")
                        nc.vector.tensor_tensor(g[:], hsq[:], t1[:], ALU.mult)
                        gsq = work.tile([128, 4 * DH], F16, tag="gsq",
                                        name="gsq")
                        ssq2 = tiny.tile([128, 1], F32, tag="ssq2",
                                         name="ssq2")
                        nc.scalar.activation(gsq[:], g[:], ACTF.Square,
                                             accum_out=ssq2[:])
                        nc.vector.tensor_scalar(ssq2[:], ssq2[:],
                                                1.0 / (4 * DH), EPS,
                                                ALU.mult, ALU.add)
                        nc.scalar.activation(ssq2[:], ssq2[:], ACTF.Sqrt)
                        ni = tiny.tile([128, 1], F32, tag="ni", name="ni")
                        nc.vector.reciprocal(ni[:], ssq2[:])
                        nsc = tiny.tile([128, 1], F32, tag="nsc", name="nsc")
                        nc.vector.tensor_scalar(nsc[:], ni[:], SIG_C, None,
                                                ALU.mult)
                        sig = work.tile([128, 4 * DH], F16, tag="sig",
                                        name="sig")
                        nc.scalar.activation(sig[:], g[:], ACTF.Sigmoid,
                                             scale=nsc[:])
                        u = work.tile([128, 4 * DH], F16, tag="u", name="u")
                        nc.vector.scalar_tensor_tensor(u[:], g[:], ni[:],
                                                       sig[:], ALU.mult,
                                                       ALU.mult)
                        ot_ps = ps_e3.tile([DH, 128], F32, tag="ot_ps",
                                           name="ot_ps")
                        for ub in range(2):
                            us = slice(ub * 128, (ub + 1) * 128)
                            ut_ps = ps_e2.tile([128, 128], F16, tag="sm",
                                               name="ut_ps")
                            nc.tensor.transpose(ut_ps[:], u[:, us],
                                                ident16[:])
                            utsb = work.tile([128, 128], F16, tag="utsb",
                                             name="utsb")
                            nc.scalar.copy(utsb[:], ut_ps[:])
                            nc.tensor.matmul(ot_ps[:], pjw_b[ub][:], utsb[:],
                                             start=(ub == 0), stop=False)
                        rzb = tiny.tile([128, 1], F16, tag="rzb", name="rzb")
                        nc.vector.tensor_copy(rzb[:], rz[:])
                        rzt_ps = ps_e2.tile([1, 128], F16, tag="sm",
                                            name="rzt_ps")
                        nc.tensor.transpose(rzt_ps[:], rzb[:], ident16[:])
                        rzrow = tiny.tile([1, 128], F16, tag="rzrow",
                                          name="rzrow")
                        nc.scalar.copy(rzrow[:], rzt_ps[:])
                        nc.tensor.matmul(ot_ps[:], pjb_b[:], ones16[:, 0:128],
                                         start=False, stop=False)
                        nc.tensor.matmul(ot_ps[:], vsink_b[:, hsl], rzrow[:],
                                         start=False, stop=True)
                        nc.scalar.copy(ctx_slab[pair][rs_, ts_], ot_ps[:])

            # ---------------- stage F: output projection + RS ----------
            with (
                tc.tile_pool(name="ps_f", bufs=2, space="PSUM") as ps_f,
                tc.tile_pool(name="fpool", bufs=2) as fpool,
            ):
                for ob in range(NCH):
                    obs = slice(ob * 128, (ob + 1) * 128)
                    y_ps = ps_f.tile([128, T], F32, tag="y_ps", name="y_ps")
                    for p in range(NPAIR):
                        for nh in range(2):
                            ns = slice(nh * 512, (nh + 1) * 512)
                            nc.tensor.matmul(y_ps[:, ns], wop_b[p][:, obs],
                                             ctx_slab[p][:, ns],
                                             start=(p == 0), stop=False)
                    for nh in range(2):
                        ns = slice(nh * 512, (nh + 1) * 512)
                        nc.tensor.matmul(y_ps[:, ns], yb_b[:, obs],
                                         ones16[:, ns],
                                         start=False, stop=True)
                    y_sb = fpool.tile([128, T], F32, tag="y_sb", name="y_sb")
                    nc.scalar.copy(y_sb[:], y_ps[:])
                    nc.sync.dma_start(y_bounce[obs, :], y_sb[:])
                nc.gpsimd.collective_compute(
                    "ReduceScatter", ALU.add,
                    ins=[y_bounce.ap().opt()],
                    outs=[y_rs.ap().opt()],
                    replica_groups=[list(range(N_CORES))],
                )
                rs_sb = fpool.tile([C // N_CORES, T], F32, tag="y_sb",
                                   name="rs_sb")
                nc.sync.dma_start(rs_sb[:], y_rs[:, :])
                o16 = fpool.tile([C // N_CORES, T], F16, tag="o16",
                                 name="o16")
                nc.vector.tensor_copy(o16[:], rs_sb[:])
                nc.sync.dma_start(out[:, :], o16[:])
    nc.finalize()
    return nc


_NC_CACHE = {}


def kernel(**inputs):
    in_maps = _host_prep(**inputs)
    if "nc" not in _NC_CACHE:
        _NC_CACHE["nc"] = build_kernel()
    nc = _NC_CACHE["nc"]
    res = run_bass_kernel_spmd(nc, in_maps, core_ids=list(range(N_CORES)))
    slabs = [res.results[c]["out"] for c in range(N_CORES)]
    yT = np.concatenate(slabs, axis=0).astype(np.float32)
    return np.ascontiguousarray(yT.T)[None]
